# revision 15
# baseline (speedup 1.0000x reference)
"""Trainium2 Bass kernel for nn_DecoderLSTM (2-layer LSTM decoder step + vocab head).

Sharding (8 cores, tensor-parallel):
  - LSTM gate weights sharded on the 4*hidden gate dim: core j owns hidden
    slice j*128..(j+1)*128 of each of the i/f/g/o gates (512 gate rows/core).
    Layer-0 needs no communication (x and h0 are replicated); h1/h2 slices are
    AllGathered (4KB) between layers.
  - W_out / b_out sharded row-wise (vocab dim): core j owns 6400 padded vocab
    rows.  log_softmax is computed shard-wise: each core computes
    S_j = sum(exp(logits_j)), one AllGather of the scalars, then
    logp = logits - log(sum_j S_j).  (Logits here are ~[-1,1] so the max
    subtraction is unnecessary for fp32 numerics; pad rows get bias -1e4 so
    exp()==0.)
  - emb is replicated; the single row is fetched on-device with a
    register-offset dynamic DMA, then ReLU'd.

Weight layout prep happens on the host in kernel(): gate-row slices are
pre-transposed so the contraction (K) dim lands on SBUF partitions, and W_out
is pre-transposed/padded per core.
"""

import numpy as np

HID = 1024
NK = HID // 128          # 8 contraction chunks of 128
GATE = 512               # 4 gates x 128 rows per core
VOCAB = 50257
VLOC = 6400              # padded vocab rows per core (50 x 128)
VPAD = VLOC * 8
N_CORES = 8
NT = (VLOC + 511) // 512  # logits tiles of 512
PAD_BIAS = -10000.0      # pad logit -> exp() == 0

# dtype knobs (perf vs precision): fp16 = half DMA + 1 cycle/row on PE, and
# ~8x less rounding error than bf16 for these small-range weights.
GATE_HALF = True         # stream LSTM gate weights in fp16
WOUT_HALF = True         # stream W_out in fp16 (halves the DMA bottleneck)

_CACHE = {}


def _build_program():
    from concourse import bacc, bass, mybir, tile

    f32 = mybir.dt.float32
    f16 = mybir.dt.float16
    u32 = mybir.dt.uint32
    AF = mybir.ActivationFunctionType

    wout_dt = f16 if WOUT_HALF else f32
    gate_dt = f16 if GATE_HALF else f32

    nc = bacc.Bacc(
        "TRN2",
        target_bir_lowering=False,
        debug=False,
        num_devices=N_CORES,
    )

    # ---- external inputs (per core) ----
    emb = nc.dram_tensor("emb", [VOCAB, HID], f32, kind="ExternalInput")
    widx = nc.dram_tensor("widx", [1, 1], u32, kind="ExternalInput")
    h0a = nc.dram_tensor("h0a", [2 * NK, 128], gate_dt, kind="ExternalInput")
    h0b = nc.dram_tensor("h0b", [2 * NK, 128], gate_dt, kind="ExternalInput")
    c0a = nc.dram_tensor("c0a", [1, 128], f32, kind="ExternalInput")
    c0b = nc.dram_tensor("c0b", [1, 128], f32, kind="ExternalInput")
    # gate weights pre-tiled: block q holds k-chunks 4q..4q+3 side by side
    # ([128, 4*GATE] each, 4KB DMA lines) of [W_ih_slice.T ; W_hh_slice.T]
    w0T = nc.dram_tensor("w0T", [4, 128, 4 * GATE], gate_dt, kind="ExternalInput")
    w1T = nc.dram_tensor("w1T", [4, 128, 4 * GATE], gate_dt, kind="ExternalInput")
    b0 = nc.dram_tensor("b0", [1, GATE], f32, kind="ExternalInput")
    b1 = nc.dram_tensor("b1", [1, GATE], f32, kind="ExternalInput")
    # W_out pre-tiled: block t = vocab group t (512 rows), all 8 k-chunks
    # side by side -> [128, 4096] contiguous (8KB DMA lines)
    woutT = nc.dram_tensor("woutT", [NT, 128, NK * 512], wout_dt, kind="ExternalInput")
    bout = nc.dram_tensor("bout", [1, VLOC], f32, kind="ExternalInput")

    # ---- external outputs (per core) ----
    h1_out = nc.dram_tensor("h1_out", [1, 128], f32, kind="ExternalOutput")
    c1_out = nc.dram_tensor("c1_out", [1, 128], f32, kind="ExternalOutput")
    h2_out = nc.dram_tensor("h2_out", [1, 128], f32, kind="ExternalOutput")
    c2_out = nc.dram_tensor("c2_out", [1, 128], f32, kind="ExternalOutput")
    logp_out = nc.dram_tensor("logp_out", [1, VLOC], f32, kind="ExternalOutput")

    # ---- internal DRAM for collectives ----
    h1_cc_in = nc.dram_tensor("h1_cc_in", [1, 256], gate_dt)
    h1_cc_out = nc.dram_tensor("h1_cc_out", [NK, 256], gate_dt, addr_space="Shared")
    h2_cc_in = nc.dram_tensor("h2_cc_in", [1, 128], wout_dt)
    h2_cc_out = nc.dram_tensor("h2_cc_out", [NK, 128], wout_dt, addr_space="Shared")
    s_cc_in = nc.dram_tensor("s_cc_in", [1, 8], f32)
    s_cc_out = nc.dram_tensor("s_cc_out", [8, 8], f32, addr_space="Shared")

    RG = [list(range(N_CORES))]

    with tile.TileContext(nc) as tc:
        with (
            tc.tile_pool(name="sp", bufs=1) as sp,
            tc.tile_pool(name="wg", bufs=8) as wg,
            tc.tile_pool(name="wo", bufs=10) as wo,
            tc.tile_pool(name="ps", bufs=1, space="PSUM") as ps,
            tc.tile_pool(name="pslg", bufs=4, space="PSUM") as pslg,
        ):
            # ---------- embedding row fetch + ReLU ----------
            widx_reg = nc.alloc_registers(
                "widx_reg", bass.OrderedSet([mybir.EngineType.Pool])
            )
            nc.reg_load(widx_reg.handles[0], widx.ap()[0:1, 0:1])
            off = nc.snap(widx_reg, donate=True, min_val=0, max_val=VOCAB - 1)

            emb3 = emb.ap().rearrange("v (m p) -> v m p", p=128)
            x_raw = sp.tile([128, NK], f32, name="x_raw")
            # src [1, NK, 128] iterated (m, p); dst iterated (p, m) -> transpose AP
            nc.gpsimd.dma_start(
                x_raw, emb3[bass.ds(off, 1), :, :].rearrange("o m p -> (o p) m")
            )

            # xh0x = relu(x); xh0h = h0[0] hi (cols 0..7) + lo (cols 8..15)
            xh0x = sp.tile([128, NK], gate_dt, name="xh0x")
            nc.scalar.activation(xh0x, x_raw, AF.Relu)
            xh0h = sp.tile([128, 2 * NK], gate_dt, name="xh0h")
            nc.gpsimd.dma_start(xh0h, h0a.ap().rearrange("m p -> p m"))

            b0_sb = sp.tile([1, GATE], f32, name="b0_sb")
            nc.gpsimd.dma_start(b0_sb, b0.ap())
            b1_sb = sp.tile([1, GATE], f32, name="b1_sb")
            nc.gpsimd.dma_start(b1_sb, b1.ap())
            c0a_sb = sp.tile([1, 128], f32, name="c0a_sb")
            nc.gpsimd.dma_start(c0a_sb, c0a.ap())
            c0b_sb = sp.tile([1, 128], f32, name="c0b_sb")
            nc.gpsimd.dma_start(c0b_sb, c0b.ap())
            bout_sb = sp.tile([1, VLOC], f32, name="bout_sb")
            nc.gpsimd.dma_start(bout_sb, bout.ap())

            def lstm_layer(
                lhs_cols, wT, b_sb, c_prev, h_out_d, c_out_d, h_cc_in_d, send_lo, tag
            ):
                """Gate matmuls + nonlinearity for one layer (this core's
                128-slice of the hidden state).  lhs_cols[m] = list of lhsT
                column APs (hi, and optionally lo) for weight k-chunk m.
                Gate row order is [i, f, o, g] (one fused sigmoid).  The
                h slice is cast to fp16 (hi, + lo residual when send_lo) and
                written to h_cc_in_d for the AllGather."""
                g_ps = ps.tile([1, GATE], f32, name=f"g_ps{tag}")
                blocks = []
                for q in range(4):
                    wt = wg.tile([128, 4 * GATE], gate_dt, name="wt_g", tag="wt_g")
                    nc.sync.dma_start(wt, wT.ap()[q])
                    blocks.append(wt)
                n_mm = sum(len(c) for c in lhs_cols)
                i_mm = 0
                for m in range(2 * NK):
                    wslc = blocks[m // 4][:, (m % 4) * GATE : (m % 4 + 1) * GATE]
                    for col in lhs_cols[m]:
                        nc.tensor.matmul(
                            g_ps,
                            col,
                            wslc,
                            start=(i_mm == 0),
                            stop=(i_mm == n_mm - 1),
                        )
                        i_mm += 1
                gates = sp.tile([1, GATE], f32, name=f"gates{tag}")
                nc.vector.tensor_add(gates, g_ps, b_sb)
                act = sp.tile([1, GATE], f32, name=f"act{tag}")
                nc.scalar.activation(act[:, 0:384], gates[:, 0:384], AF.Sigmoid)
                nc.scalar.activation(act[:, 384:512], gates[:, 384:512], AF.Tanh)
                t_fc = sp.tile([1, 128], f32, name=f"t_fc{tag}")
                nc.vector.tensor_mul(t_fc, act[:, 128:256], c_prev)
                t_ig = sp.tile([1, 128], f32, name=f"t_ig{tag}")
                nc.vector.tensor_mul(t_ig, act[:, 0:128], act[:, 384:512])
                c_new = sp.tile([1, 128], f32, name=f"c_new{tag}")
                nc.vector.tensor_add(c_new, t_fc, t_ig)
                tanh_c = sp.tile([1, 128], f32, name=f"tanh_c{tag}")
                nc.scalar.activation(tanh_c, c_new, AF.Tanh)
                h_new = sp.tile([1, 128], f32, name=f"h_new{tag}")
                nc.vector.tensor_mul(h_new, act[:, 256:384], tanh_c)
                npay = 256 if send_lo else 128
                hpay = sp.tile([1, npay], wout_dt if not send_lo else gate_dt,
                               name=f"hpay{tag}")
                nc.vector.tensor_copy(hpay[:, 0:128], h_new)
                if send_lo:
                    hi32 = sp.tile([1, 128], f32, name=f"hi32{tag}")
                    nc.vector.tensor_copy(hi32, hpay[:, 0:128])
                    nc.vector.tensor_sub(hpay[:, 128:256], h_new, hi32)
                nc.scalar.dma_start(h_cc_in_d.ap(), hpay)
                nc.gpsimd.dma_start(c_out_d.ap(), c_new)
                nc.gpsimd.dma_start(h_out_d.ap(), h_new)
                return h_new, c_new

            # ---------- layer 0 ----------
            lhs0 = [[xh0x[:, m : m + 1]] for m in range(NK)] + [
                [xh0h[:, m : m + 1], xh0h[:, m + NK : m + NK + 1]] for m in range(NK)
            ]
            lstm_layer(lhs0, w0T, b0_sb, c0a_sb, h1_out, c1_out, h1_cc_in, True, "0")
            nc.gpsimd.collective_compute(
                "AllGather",
                mybir.AluOpType.bypass,
                replica_groups=RG,
                ins=[h1_cc_in.ap()],
                outs=[h1_cc_out.ap()],
            )

            # ---------- layer 1 ----------
            # xh1 cols: 0..7 h1 hi; 8..15 h0[1] hi; 16..23 h0[1] lo; 24..31 h1 lo
            xh1 = sp.tile([128, 4 * NK], gate_dt, name="xh1")
            nc.scalar.dma_start(
                xh1[:, 0:NK], h1_cc_out.ap()[:, 0:128].rearrange("m p -> p m")
            )
            nc.scalar.dma_start(
                xh1[:, 3 * NK : 4 * NK],
                h1_cc_out.ap()[:, 128:256].rearrange("m p -> p m"),
            )
            nc.gpsimd.dma_start(xh1[:, NK : 3 * NK], h0b.ap().rearrange("m p -> p m"))
            lhs1 = [[xh1[:, m : m + 1], xh1[:, 3 * NK + m : 3 * NK + m + 1]]
                    for m in range(NK)] + [
                [xh1[:, NK + m : NK + m + 1], xh1[:, 2 * NK + m : 2 * NK + m + 1]]
                for m in range(NK)
            ]
            lstm_layer(lhs1, w1T, b1_sb, c0b_sb, h2_out, c2_out, h2_cc_in, False, "1")
            nc.gpsimd.collective_compute(
                "AllGather",
                mybir.AluOpType.bypass,
                replica_groups=RG,
                ins=[h2_cc_in.ap()],
                outs=[h2_cc_out.ap()],
            )

            # full h2 (fp16) -> [128, 8], chunk m in col m
            h2c = sp.tile([128, NK], wout_dt, name="h2c")
            nc.scalar.dma_start(h2c, h2_cc_out.ap().rearrange("m p -> p m"))

            # ---------- logits matvec + exp partial sums ----------
            logits = sp.tile([1, VLOC], f32, name="logits")
            s_parts = sp.tile([1, NT], f32, name="s_parts")
            for t in range(NT):
                n0 = t * 512
                nn = min(512, VLOC - n0)
                wt = wo.tile([128, NK * 512], wout_dt, name="wt_o", tag="wt_o")
                eng = nc.sync
                eng.dma_start(wt[:, : NK * nn], woutT.ap()[t, :, : NK * nn])
                lg_ps = pslg.tile([1, 512], f32, name="lg_ps", tag="lg")
                for m in range(NK):
                    nc.tensor.matmul(
                        lg_ps[:, :nn],
                        h2c[:, m : m + 1],
                        wt[:, m * nn : (m + 1) * nn],
                        start=(m == 0),
                        stop=(m == NK - 1),
                    )
                nc.vector.tensor_add(
                    logits[:, n0 : n0 + nn], lg_ps[:, :nn], bout_sb[:, n0 : n0 + nn]
                )
                exp_scr = sp.tile([1, 512], f32, name="exp_scr", tag="exp_scr", bufs=2)
                nc.scalar.activation(
                    exp_scr[:, :nn],
                    logits[:, n0 : n0 + nn],
                    AF.Exp,
                    accum_out=s_parts[:, t : t + 1],
                )

            # ---------- sharded log-softmax stats ----------
            s_sum = sp.tile([1, 1], f32, name="s_sum")
            nc.vector.reduce_sum(s_sum, s_parts, axis=mybir.AxisListType.X)
            s_bcast = sp.tile([1, 8], f32, name="s_bcast")
            nc.vector.tensor_copy(s_bcast, s_sum.to_broadcast((1, 8)))
            nc.scalar.dma_start(s_cc_in.ap(), s_bcast)
            nc.gpsimd.collective_compute(
                "AllGather",
                mybir.AluOpType.bypass,
                replica_groups=RG,
                ins=[s_cc_in.ap()],
                outs=[s_cc_out.ap()],
            )
            s_col = sp.tile([8, 1], f32, name="s_col")
            nc.scalar.dma_start(s_col, s_cc_out.ap()[:, 0:1])
            ones8 = sp.tile([8, 128], f32, name="ones8")
            nc.vector.memset(ones8, 1.0)
            S_ps = ps.tile([128, 1], f32, name="S_ps")
            nc.tensor.matmul(S_ps, ones8, s_col, start=True, stop=True)
            logS = sp.tile([1, 1], f32, name="logS")
            nc.scalar.activation(logS, S_ps[0:1, :], AF.Ln)
            neglogS = sp.tile([1, 1], f32, name="neglogS")
            nc.scalar.mul(neglogS, logS, -1.0)

            # logp = logits - logS (split across ACT and DVE), in-place
            half = 2816
            nc.scalar.activation(
                logits[:, 0:half], logits[:, 0:half], AF.Identity, bias=neglogS
            )
            nc.vector.tensor_scalar_add(logits[:, half:], logits[:, half:], neglogS)
            nc.scalar.dma_start(logp_out.ap(), logits)

    nc.compile()
    return nc


def _get_program():
    if "nc" not in _CACHE:
        _CACHE["nc"] = _build_program()
    return _CACHE["nc"]


def _pack_gate_w(Wih_rows, Whh_rows, gate_np):
    """[GATE, HID] x2 -> [4, 128, 4*GATE] pre-tiled blocks: block q holds
    k-chunks 4q..4q+3 of [Wih.T ; Whh.T] side by side."""
    wT = np.concatenate([Wih_rows.T, Whh_rows.T], axis=0).astype(gate_np)  # [2H, GATE]
    out = np.zeros((4, 128, 4 * GATE), dtype=gate_np)
    for m in range(16):
        out[m // 4, :, (m % 4) * GATE : (m % 4 + 1) * GATE] = wT[
            m * 128 : (m + 1) * 128
        ]
    return out


def _hilo_chunks(v, gate_np):
    """[HID] f32 -> [2*NK, 128]: rows 0..NK-1 = hi chunks, NK.. = lo chunks."""
    hi = v.astype(gate_np)
    lo = (v - hi.astype(np.float32)).astype(gate_np)
    return np.ascontiguousarray(
        np.concatenate([hi.reshape(NK, 128), lo.reshape(NK, 128)], axis=0)
    )


def _prep_inputs(inputs):
    """Host-side sharding: slice + transpose weights per core."""
    wout_np = np.float16 if WOUT_HALF else np.float32
    gate_np = np.float16 if GATE_HALF else np.float32

    f = lambda k: np.asarray(inputs[k], dtype=np.float32)
    emb = f("emb")
    h0, c0 = f("h0"), f("c0")
    W_ih = [f("W_ih0"), f("W_ih1")]
    W_hh = [f("W_hh0"), f("W_hh1")]
    b = [
        (f("b_ih0") + f("b_hh0")).astype(np.float32),
        (f("b_ih1") + f("b_hh1")).astype(np.float32),
    ]
    W_out, b_out = f("W_out"), f("b_out")
    idx = np.uint32(np.asarray(inputs["word_input"]).reshape(-1)[0])

    in_maps = []
    for j in range(N_CORES):
        rows = np.concatenate(
            [np.arange(g * HID + j * 128, g * HID + (j + 1) * 128) for g in (0, 1, 3, 2)]
        )
        w0T = _pack_gate_w(W_ih[0][rows], W_hh[0][rows], gate_np)
        w1T = _pack_gate_w(W_ih[1][rows], W_hh[1][rows], gate_np)
        v0 = j * VLOC
        nvalid = max(0, min(VLOC, VOCAB - v0))
        wsh = np.zeros((HID, VLOC), dtype=wout_np)
        wsh[:, :nvalid] = W_out[v0 : v0 + nvalid].T.astype(wout_np)
        woutT = np.zeros((NT, 128, NK * 512), dtype=wout_np)
        for t in range(NT):
            nn = min(512, VLOC - t * 512)
            for m in range(NK):
                woutT[t, :, m * nn : (m + 1) * nn] = wsh[
                    m * 128 : (m + 1) * 128, t * 512 : t * 512 + nn
                ]
        boutj = np.full((1, VLOC), PAD_BIAS, dtype=np.float32)
        boutj[0, :nvalid] = b_out[v0 : v0 + nvalid]
        in_maps.append(
            {
                "emb": emb,
                "widx": np.array([[idx]], dtype=np.uint32),
                "h0a": _hilo_chunks(h0[0].reshape(HID), gate_np),
                "h0b": _hilo_chunks(h0[1].reshape(HID), gate_np),
                "c0a": np.ascontiguousarray(c0[0, 0, j * 128 : (j + 1) * 128]).reshape(
                    1, 128
                ),
                "c0b": np.ascontiguousarray(c0[1, 0, j * 128 : (j + 1) * 128]).reshape(
                    1, 128
                ),
                "w0T": w0T,
                "w1T": w1T,
                "b0": b[0][rows].reshape(1, GATE).copy(),
                "b1": b[1][rows].reshape(1, GATE).copy(),
                "woutT": woutT,
                "bout": boutj,
            }
        )
    return in_maps


def _assemble(results):
    h1 = np.concatenate([r["h1_out"].reshape(128) for r in results])
    h2 = np.concatenate([r["h2_out"].reshape(128) for r in results])
    c1 = np.concatenate([r["c1_out"].reshape(128) for r in results])
    c2 = np.concatenate([r["c2_out"].reshape(128) for r in results])
    logp = np.concatenate([r["logp_out"].reshape(VLOC) for r in results])[:VOCAB]
    h_out = np.stack([h1, h2]).reshape(2, 1, HID).astype(np.float32)
    c_out = np.stack([c1, c2]).reshape(2, 1, HID).astype(np.float32)
    return logp.reshape(1, VOCAB).astype(np.float32), (h_out, c_out)


def kernel(**inputs):
    from concourse.bass_utils import run_bass_kernel_spmd

    nc = _get_program()
    in_maps = _prep_inputs(inputs)
    res = run_bass_kernel_spmd(nc, in_maps, core_ids=list(range(N_CORES)))
    return _assemble(res.results)


# revision 17
# speedup vs baseline: 1.0082x; 1.0082x over previous
"""Trainium2 Bass kernel for nn_DecoderLSTM (2-layer LSTM decoder step + vocab head).

Sharding (8 cores, tensor-parallel):
  - LSTM gate weights sharded on the 4*hidden gate dim: core j owns hidden
    slice j*128..(j+1)*128 of each of the i/f/g/o gates (512 gate rows/core).
    Layer-0 needs no communication (x and h0 are replicated); h1/h2 slices are
    AllGathered (4KB) between layers.
  - W_out / b_out sharded row-wise (vocab dim): core j owns 6400 padded vocab
    rows.  log_softmax is computed shard-wise: each core computes
    S_j = sum(exp(logits_j)), one AllGather of the scalars, then
    logp = logits - log(sum_j S_j).  (Logits here are ~[-1,1] so the max
    subtraction is unnecessary for fp32 numerics; pad rows get bias -1e4 so
    exp()==0.)
  - emb is replicated; the single row is fetched on-device with a
    register-offset dynamic DMA, then ReLU'd.

Weight layout prep happens on the host in kernel(): gate-row slices are
pre-transposed so the contraction (K) dim lands on SBUF partitions, and W_out
is pre-transposed/padded per core.
"""

import numpy as np

HID = 1024
NK = HID // 128          # 8 contraction chunks of 128
GATE = 512               # 4 gates x 128 rows per core
VOCAB = 50257
VLOC = 6400              # padded vocab rows per core (50 x 128)
VPAD = VLOC * 8
N_CORES = 8
NT = (VLOC + 511) // 512  # logits tiles of 512
PAD_BIAS = -10000.0      # pad logit -> exp() == 0

# dtype knobs (perf vs precision): fp16 = half DMA + 1 cycle/row on PE, and
# ~8x less rounding error than bf16 for these small-range weights.
GATE_HALF = True         # stream LSTM gate weights in fp16
WOUT_HALF = True         # stream W_out in fp16 (halves the DMA bottleneck)

_CACHE = {}


def _build_program():
    from concourse import bacc, bass, mybir, tile

    f32 = mybir.dt.float32
    f16 = mybir.dt.float16
    u32 = mybir.dt.uint32
    AF = mybir.ActivationFunctionType

    wout_dt = f16 if WOUT_HALF else f32
    gate_dt = f16 if GATE_HALF else f32

    nc = bacc.Bacc(
        "TRN2",
        target_bir_lowering=False,
        debug=False,
        num_devices=N_CORES,
    )

    # ---- external inputs (per core) ----
    emb = nc.dram_tensor("emb", [VOCAB, HID], f32, kind="ExternalInput")
    widx = nc.dram_tensor("widx", [1, 1], u32, kind="ExternalInput")
    h0a = nc.dram_tensor("h0a", [2 * NK, 128], gate_dt, kind="ExternalInput")
    h0b = nc.dram_tensor("h0b", [2 * NK, 128], gate_dt, kind="ExternalInput")
    c0a = nc.dram_tensor("c0a", [1, 128], f32, kind="ExternalInput")
    c0b = nc.dram_tensor("c0b", [1, 128], f32, kind="ExternalInput")
    # gate weights pre-tiled: block q holds k-chunks 4q..4q+3 side by side
    # ([128, 4*GATE] each, 4KB DMA lines) of [W_ih_slice.T ; W_hh_slice.T]
    w0T = nc.dram_tensor("w0T", [4, 128, 4 * GATE], gate_dt, kind="ExternalInput")
    w1T = nc.dram_tensor("w1T", [4, 128, 4 * GATE], gate_dt, kind="ExternalInput")
    b0 = nc.dram_tensor("b0", [1, GATE], f32, kind="ExternalInput")
    b1 = nc.dram_tensor("b1", [1, GATE], f32, kind="ExternalInput")
    # W_out pre-tiled: block t = vocab group t (512 rows), all 8 k-chunks
    # side by side -> [128, 4096] contiguous (8KB DMA lines)
    woutT = nc.dram_tensor("woutT", [NT, 128, NK * 512], wout_dt, kind="ExternalInput")
    bout = nc.dram_tensor("bout", [1, VLOC], f32, kind="ExternalInput")

    # ---- external outputs (per core) ----
    h1_out = nc.dram_tensor("h1_out", [1, 128], f32, kind="ExternalOutput")
    c1_out = nc.dram_tensor("c1_out", [1, 128], f32, kind="ExternalOutput")
    h2_out = nc.dram_tensor("h2_out", [1, 128], f32, kind="ExternalOutput")
    c2_out = nc.dram_tensor("c2_out", [1, 128], f32, kind="ExternalOutput")
    logp_out = nc.dram_tensor("logp_out", [1, VLOC], f32, kind="ExternalOutput")

    # ---- internal DRAM for collectives ----
    h1_cc_in = nc.dram_tensor("h1_cc_in", [1, 256], gate_dt)
    h1_cc_out = nc.dram_tensor("h1_cc_out", [NK, 256], gate_dt, addr_space="Shared")
    h2_cc_in = nc.dram_tensor("h2_cc_in", [1, 128], wout_dt)
    h2_cc_out = nc.dram_tensor("h2_cc_out", [NK, 128], wout_dt, addr_space="Shared")
    s_cc_in = nc.dram_tensor("s_cc_in", [1, 8], f32)
    s_cc_out = nc.dram_tensor("s_cc_out", [8, 8], f32, addr_space="Shared")

    RG = [list(range(N_CORES))]

    with tile.TileContext(nc) as tc:
        with (
            tc.tile_pool(name="sp", bufs=1) as sp,
            tc.tile_pool(name="wg", bufs=8) as wg,
            tc.tile_pool(name="wo", bufs=10) as wo,
            tc.tile_pool(name="ps", bufs=1, space="PSUM") as ps,
            tc.tile_pool(name="pslg", bufs=4, space="PSUM") as pslg,
        ):
            # ---------- embedding row fetch + ReLU ----------
            widx_reg = nc.alloc_registers(
                "widx_reg", bass.OrderedSet([mybir.EngineType.Pool])
            )
            nc.reg_load(widx_reg.handles[0], widx.ap()[0:1, 0:1])
            off = nc.snap(widx_reg, donate=True, min_val=0, max_val=VOCAB - 1)

            emb3 = emb.ap().rearrange("v (m p) -> v m p", p=128)
            x_raw = sp.tile([128, NK], f32, name="x_raw")
            # src [1, NK, 128] iterated (m, p); dst iterated (p, m) -> transpose AP
            nc.gpsimd.dma_start(
                x_raw, emb3[bass.ds(off, 1), :, :].rearrange("o m p -> (o p) m")
            )

            # xh0x = relu(x); xh0h = h0[0] hi (cols 0..7) + lo (cols 8..15)
            xh0x = sp.tile([128, NK], gate_dt, name="xh0x")
            nc.scalar.activation(xh0x, x_raw, AF.Relu)
            xh0h = sp.tile([128, 2 * NK], gate_dt, name="xh0h")
            nc.gpsimd.dma_start(xh0h, h0a.ap().rearrange("m p -> p m"))

            b0_sb = sp.tile([1, GATE], f32, name="b0_sb")
            nc.gpsimd.dma_start(b0_sb, b0.ap())
            b1_sb = sp.tile([1, GATE], f32, name="b1_sb")
            nc.gpsimd.dma_start(b1_sb, b1.ap())
            c0a_sb = sp.tile([1, 128], f32, name="c0a_sb")
            nc.gpsimd.dma_start(c0a_sb, c0a.ap())
            c0b_sb = sp.tile([1, 128], f32, name="c0b_sb")
            nc.gpsimd.dma_start(c0b_sb, c0b.ap())
            bout_sb = sp.tile([1, VLOC], f32, name="bout_sb")
            nc.gpsimd.dma_start(bout_sb, bout.ap())

            def lstm_layer(
                lhs_cols, wT, b_sb, c_prev, h_out_d, c_out_d, h_cc_in_d, send_lo, tag
            ):
                """Gate matmuls + nonlinearity for one layer (this core's
                128-slice of the hidden state).  lhs_cols[m] = list of lhsT
                column APs (hi, and optionally lo) for weight k-chunk m.
                Gate row order is [i, f, o, g] (one fused sigmoid).  The
                h slice is cast to fp16 (hi, + lo residual when send_lo) and
                written to h_cc_in_d for the AllGather."""
                g_ps = ps.tile([1, GATE], f32, name=f"g_ps{tag}")
                blocks = []
                for q in range(4):
                    wt = wg.tile([128, 4 * GATE], gate_dt, name="wt_g", tag="wt_g")
                    nc.sync.dma_start(wt, wT.ap()[q])
                    blocks.append(wt)
                n_mm = sum(len(c) for c in lhs_cols)
                i_mm = 0
                for m in range(2 * NK):
                    wslc = blocks[m // 4][:, (m % 4) * GATE : (m % 4 + 1) * GATE]
                    for col in lhs_cols[m]:
                        nc.tensor.matmul(
                            g_ps,
                            col,
                            wslc,
                            start=(i_mm == 0),
                            stop=(i_mm == n_mm - 1),
                        )
                        i_mm += 1
                gates = sp.tile([1, GATE], f32, name=f"gates{tag}")
                nc.vector.tensor_add(gates, g_ps, b_sb)
                act = sp.tile([1, GATE], f32, name=f"act{tag}")
                nc.scalar.activation(act[:, 0:384], gates[:, 0:384], AF.Sigmoid)
                nc.scalar.activation(act[:, 384:512], gates[:, 384:512], AF.Tanh)
                t_fc = sp.tile([1, 128], f32, name=f"t_fc{tag}")
                nc.vector.tensor_mul(t_fc, act[:, 128:256], c_prev)
                t_ig = sp.tile([1, 128], f32, name=f"t_ig{tag}")
                nc.vector.tensor_mul(t_ig, act[:, 0:128], act[:, 384:512])
                c_new = sp.tile([1, 128], f32, name=f"c_new{tag}")
                nc.vector.tensor_add(c_new, t_fc, t_ig)
                tanh_c = sp.tile([1, 128], f32, name=f"tanh_c{tag}")
                nc.scalar.activation(tanh_c, c_new, AF.Tanh)
                h_new = sp.tile([1, 128], f32, name=f"h_new{tag}")
                nc.vector.tensor_mul(h_new, act[:, 256:384], tanh_c)
                npay = 256 if send_lo else 128
                hpay = sp.tile([1, npay], wout_dt if not send_lo else gate_dt,
                               name=f"hpay{tag}")
                nc.vector.tensor_copy(hpay[:, 0:128], h_new)
                if send_lo:
                    hi32 = sp.tile([1, 128], f32, name=f"hi32{tag}")
                    nc.vector.tensor_copy(hi32, hpay[:, 0:128])
                    nc.vector.tensor_sub(hpay[:, 128:256], h_new, hi32)
                nc.scalar.dma_start(h_cc_in_d.ap(), hpay)
                nc.gpsimd.dma_start(c_out_d.ap(), c_new)
                nc.gpsimd.dma_start(h_out_d.ap(), h_new)
                return h_new, c_new

            # ---------- layer 0 ----------
            lhs0 = [[xh0x[:, m : m + 1]] for m in range(NK)] + [
                [xh0h[:, m : m + 1], xh0h[:, m + NK : m + NK + 1]] for m in range(NK)
            ]
            lstm_layer(lhs0, w0T, b0_sb, c0a_sb, h1_out, c1_out, h1_cc_in, True, "0")
            nc.gpsimd.collective_compute(
                "AllGather",
                mybir.AluOpType.bypass,
                replica_groups=RG,
                ins=[h1_cc_in.ap()],
                outs=[h1_cc_out.ap()],
            )

            # ---------- layer 1 ----------
            # xh1 cols: 0..7 h1 hi; 8..15 h0[1] hi; 16..23 h0[1] lo; 24..31 h1 lo
            xh1 = sp.tile([128, 4 * NK], gate_dt, name="xh1")
            nc.scalar.dma_start(
                xh1[:, 0:NK], h1_cc_out.ap()[:, 0:128].rearrange("m p -> p m")
            )
            nc.scalar.dma_start(
                xh1[:, 3 * NK : 4 * NK],
                h1_cc_out.ap()[:, 128:256].rearrange("m p -> p m"),
            )
            nc.gpsimd.dma_start(xh1[:, NK : 3 * NK], h0b.ap().rearrange("m p -> p m"))
            lhs1 = [[xh1[:, m : m + 1], xh1[:, 3 * NK + m : 3 * NK + m + 1]]
                    for m in range(NK)] + [
                [xh1[:, NK + m : NK + m + 1], xh1[:, 2 * NK + m : 2 * NK + m + 1]]
                for m in range(NK)
            ]
            lstm_layer(lhs1, w1T, b1_sb, c0b_sb, h2_out, c2_out, h2_cc_in, False, "1")
            nc.gpsimd.collective_compute(
                "AllGather",
                mybir.AluOpType.bypass,
                replica_groups=RG,
                ins=[h2_cc_in.ap()],
                outs=[h2_cc_out.ap()],
            )

            # full h2 (fp16) -> [128, 8], chunk m in col m
            h2c = sp.tile([128, NK], wout_dt, name="h2c")
            nc.scalar.dma_start(h2c, h2_cc_out.ap().rearrange("m p -> p m"))

            # ---------- logits matvec + exp partial sums ----------
            logits = sp.tile([1, VLOC], f32, name="logits")
            s_parts = sp.tile([1, NT], f32, name="s_parts")
            for t in range(NT):
                n0 = t * 512
                nn = min(512, VLOC - n0)
                wt = wo.tile([128, NK * 512], wout_dt, name="wt_o", tag="wt_o")
                eng = nc.sync
                eng.dma_start(wt[:, : NK * nn], woutT.ap()[t, :, : NK * nn])
                lg_ps = pslg.tile([1, 512], f32, name="lg_ps", tag="lg")
                for m in range(NK):
                    nc.tensor.matmul(
                        lg_ps[:, :nn],
                        h2c[:, m : m + 1],
                        wt[:, m * nn : (m + 1) * nn],
                        start=(m == 0),
                        stop=(m == NK - 1),
                    )
                nc.vector.tensor_add(
                    logits[:, n0 : n0 + nn], lg_ps[:, :nn], bout_sb[:, n0 : n0 + nn]
                )
                exp_scr = sp.tile([1, 512], f32, name="exp_scr", tag="exp_scr", bufs=2)
                nc.scalar.activation(
                    exp_scr[:, :nn],
                    logits[:, n0 : n0 + nn],
                    AF.Exp,
                    accum_out=s_parts[:, t : t + 1],
                )

            # ---------- sharded log-softmax stats ----------
            s_sum = sp.tile([1, 1], f32, name="s_sum")
            nc.vector.reduce_sum(s_sum, s_parts, axis=mybir.AxisListType.X)
            s_bcast = sp.tile([1, 8], f32, name="s_bcast")
            nc.vector.tensor_copy(s_bcast, s_sum.to_broadcast((1, 8)))
            nc.scalar.dma_start(s_cc_in.ap(), s_bcast)
            nc.gpsimd.collective_compute(
                "AllGather",
                mybir.AluOpType.bypass,
                replica_groups=RG,
                ins=[s_cc_in.ap()],
                outs=[s_cc_out.ap()],
            )
            s_col = sp.tile([8, 1], f32, name="s_col")
            nc.scalar.dma_start(s_col, s_cc_out.ap()[:, 0:1])
            ones8 = sp.tile([8, 128], f32, name="ones8")
            nc.vector.memset(ones8, 1.0)
            S_ps = ps.tile([128, 1], f32, name="S_ps")
            nc.tensor.matmul(S_ps, ones8, s_col, start=True, stop=True)
            logS = sp.tile([1, 1], f32, name="logS")
            nc.scalar.activation(logS, S_ps[0:1, :], AF.Ln)
            neglogS = sp.tile([1, 1], f32, name="neglogS")
            nc.scalar.mul(neglogS, logS, -1.0)

            # logp = logits - logS (split across ACT and DVE), in-place
            half = 2816
            nc.scalar.activation(
                logits[:, 0:half], logits[:, 0:half], AF.Identity, bias=neglogS
            )
            nc.vector.tensor_scalar_add(logits[:, half:], logits[:, half:], neglogS)
            nc.scalar.dma_start(logp_out.ap(), logits)

    nc.compile()
    return nc


def _get_program():
    if "nc" not in _CACHE:
        _CACHE["nc"] = _build_program()
    return _CACHE["nc"]


def _pack_gate_w(Wih_rows, Whh_rows, gate_np):
    """[GATE, HID] x2 -> [4, 128, 4*GATE] pre-tiled blocks: block q holds
    k-chunks 4q..4q+3 of [Wih.T ; Whh.T] side by side."""
    wT = np.concatenate([Wih_rows.T, Whh_rows.T], axis=0).astype(gate_np)  # [2H, GATE]
    out = np.zeros((4, 128, 4 * GATE), dtype=gate_np)
    for m in range(16):
        out[m // 4, :, (m % 4) * GATE : (m % 4 + 1) * GATE] = wT[
            m * 128 : (m + 1) * 128
        ]
    return out


def _hilo_chunks(v, gate_np):
    """[HID] f32 -> [2*NK, 128]: rows 0..NK-1 = hi chunks, NK.. = lo chunks."""
    hi = v.astype(gate_np)
    lo = (v - hi.astype(np.float32)).astype(gate_np)
    return np.ascontiguousarray(
        np.concatenate([hi.reshape(NK, 128), lo.reshape(NK, 128)], axis=0)
    )


def _prep_inputs(inputs):
    """Host-side sharding: slice + transpose weights per core."""
    wout_np = np.float16 if WOUT_HALF else np.float32
    gate_np = np.float16 if GATE_HALF else np.float32

    f = lambda k: np.asarray(inputs[k], dtype=np.float32)
    emb = f("emb")
    h0, c0 = f("h0"), f("c0")
    W_ih = [f("W_ih0"), f("W_ih1")]
    W_hh = [f("W_hh0"), f("W_hh1")]
    b = [
        (f("b_ih0") + f("b_hh0")).astype(np.float32),
        (f("b_ih1") + f("b_hh1")).astype(np.float32),
    ]
    W_out, b_out = f("W_out"), f("b_out")
    idx = np.uint32(np.asarray(inputs["word_input"]).reshape(-1)[0])

    in_maps = []
    for j in range(N_CORES):
        rows = np.concatenate(
            [np.arange(g * HID + j * 128, g * HID + (j + 1) * 128) for g in (0, 1, 3, 2)]
        )
        w0T = _pack_gate_w(W_ih[0][rows], W_hh[0][rows], gate_np)
        w1T = _pack_gate_w(W_ih[1][rows], W_hh[1][rows], gate_np)
        v0 = j * VLOC
        nvalid = max(0, min(VLOC, VOCAB - v0))
        wsh = np.zeros((HID, VLOC), dtype=wout_np)
        wsh[:, :nvalid] = W_out[v0 : v0 + nvalid].T.astype(wout_np)
        woutT = np.zeros((NT, 128, NK * 512), dtype=wout_np)
        for t in range(NT):
            nn = min(512, VLOC - t * 512)
            for m in range(NK):
                woutT[t, :, m * nn : (m + 1) * nn] = wsh[
                    m * 128 : (m + 1) * 128, t * 512 : t * 512 + nn
                ]
        boutj = np.full((1, VLOC), PAD_BIAS, dtype=np.float32)
        boutj[0, :nvalid] = b_out[v0 : v0 + nvalid]
        in_maps.append(
            {
                "emb": emb,
                "widx": np.array([[idx]], dtype=np.uint32),
                "h0a": _hilo_chunks(h0[0].reshape(HID), gate_np),
                "h0b": _hilo_chunks(h0[1].reshape(HID), gate_np),
                "c0a": np.ascontiguousarray(c0[0, 0, j * 128 : (j + 1) * 128]).reshape(
                    1, 128
                ),
                "c0b": np.ascontiguousarray(c0[1, 0, j * 128 : (j + 1) * 128]).reshape(
                    1, 128
                ),
                "w0T": w0T,
                "w1T": w1T,
                "b0": b[0][rows].reshape(1, GATE).copy(),
                "b1": b[1][rows].reshape(1, GATE).copy(),
                "woutT": woutT,
                "bout": boutj,
            }
        )
    return in_maps


def _assemble(results):
    h1 = np.concatenate([r["h1_out"].reshape(128) for r in results])
    h2 = np.concatenate([r["h2_out"].reshape(128) for r in results])
    c1 = np.concatenate([r["c1_out"].reshape(128) for r in results])
    c2 = np.concatenate([r["c2_out"].reshape(128) for r in results])
    logp = np.concatenate([r["logp_out"].reshape(VLOC) for r in results])[:VOCAB]
    h_out = np.stack([h1, h2]).reshape(2, 1, HID).astype(np.float32)
    c_out = np.stack([c1, c2]).reshape(2, 1, HID).astype(np.float32)
    return logp.reshape(1, VOCAB).astype(np.float32), (h_out, c_out)


def kernel(**inputs):
    from concourse.bass_utils import run_bass_kernel_spmd

    nc = _get_program()
    in_maps = _prep_inputs(inputs)
    res = run_bass_kernel_spmd(nc, in_maps, core_ids=list(range(N_CORES)))
    return _assemble(res.results)


# revision 18
# speedup vs baseline: 1.0506x; 1.0421x over previous
"""Trainium2 Bass kernel for nn_DecoderLSTM (2-layer LSTM decoder step + vocab head).

Sharding (8 cores, tensor-parallel):
  - LSTM gate weights sharded on the 4*hidden gate dim: core j owns hidden
    slice j*128..(j+1)*128 of each of the i/f/g/o gates (512 gate rows/core).
    Layer-0 needs no communication (x and h0 are replicated); h1/h2 slices are
    AllGathered (4KB) between layers.
  - W_out / b_out sharded row-wise (vocab dim): core j owns 6400 padded vocab
    rows.  log_softmax is computed shard-wise: each core computes
    S_j = sum(exp(logits_j)), one AllGather of the scalars, then
    logp = logits - log(sum_j S_j).  (Logits here are ~[-1,1] so the max
    subtraction is unnecessary for fp32 numerics; pad rows get bias -1e4 so
    exp()==0.)
  - emb is replicated; the single row is fetched on-device with a
    register-offset dynamic DMA, then ReLU'd.

Weight layout prep happens on the host in kernel(): gate-row slices are
pre-transposed so the contraction (K) dim lands on SBUF partitions, and W_out
is pre-transposed/padded per core.
"""

import numpy as np

HID = 1024
NK = HID // 128          # 8 contraction chunks of 128
GATE = 512               # 4 gates x 128 rows per core
VOCAB = 50257
VLOC = 6400              # padded vocab rows per core (50 x 128)
VPAD = VLOC * 8
N_CORES = 8
NT = (VLOC + 511) // 512  # logits tiles of 512
PAD_BIAS = -10000.0      # pad logit -> exp() == 0

# dtype knobs (perf vs precision): fp16 = half DMA + 1 cycle/row on PE, and
# ~8x less rounding error than bf16 for these small-range weights.
GATE_HALF = True         # stream LSTM gate weights in fp16
WOUT_HALF = True         # stream W_out in fp16 (halves the DMA bottleneck)

_CACHE = {}


def _build_program():
    from concourse import bacc, bass, mybir, tile

    f32 = mybir.dt.float32
    f16 = mybir.dt.float16
    u32 = mybir.dt.uint32
    AF = mybir.ActivationFunctionType

    wout_dt = f16 if WOUT_HALF else f32
    gate_dt = f16 if GATE_HALF else f32

    nc = bacc.Bacc(
        "TRN2",
        target_bir_lowering=False,
        debug=False,
        num_devices=N_CORES,
    )

    # ---- external inputs (per core) ----
    emb = nc.dram_tensor("emb", [VOCAB, HID], f32, kind="ExternalInput")
    widx = nc.dram_tensor("widx", [1, 1], u32, kind="ExternalInput")
    h0a = nc.dram_tensor("h0a", [2 * NK, 128], gate_dt, kind="ExternalInput")
    h0b = nc.dram_tensor("h0b", [2 * NK, 128], gate_dt, kind="ExternalInput")
    c0a = nc.dram_tensor("c0a", [1, 128], f32, kind="ExternalInput")
    c0b = nc.dram_tensor("c0b", [1, 128], f32, kind="ExternalInput")
    # gate weights pre-tiled: block q holds k-chunks 4q..4q+3 side by side
    # ([128, 4*GATE] each, 4KB DMA lines) of [W_ih_slice.T ; W_hh_slice.T]
    w0T = nc.dram_tensor("w0T", [4, 128, 4 * GATE], gate_dt, kind="ExternalInput")
    w1T = nc.dram_tensor("w1T", [4, 128, 4 * GATE], gate_dt, kind="ExternalInput")
    b0 = nc.dram_tensor("b0", [1, GATE], f32, kind="ExternalInput")
    b1 = nc.dram_tensor("b1", [1, GATE], f32, kind="ExternalInput")
    # W_out pre-tiled: block t = vocab group t (512 rows), all 8 k-chunks
    # side by side -> [128, 4096] contiguous (8KB DMA lines)
    woutT = nc.dram_tensor("woutT", [NT, 128, NK * 512], wout_dt, kind="ExternalInput")
    bout = nc.dram_tensor("bout", [1, VLOC], f32, kind="ExternalInput")

    # ---- external outputs (per core) ----
    h1_out = nc.dram_tensor("h1_out", [1, 128], f32, kind="ExternalOutput")
    c1_out = nc.dram_tensor("c1_out", [1, 128], f32, kind="ExternalOutput")
    h2_out = nc.dram_tensor("h2_out", [1, 128], f32, kind="ExternalOutput")
    c2_out = nc.dram_tensor("c2_out", [1, 128], f32, kind="ExternalOutput")
    logp_out = nc.dram_tensor("logp_out", [1, VLOC], f32, kind="ExternalOutput")

    # ---- internal DRAM for collectives ----
    h1_cc_in = nc.dram_tensor("h1_cc_in", [1, 128], gate_dt)
    h1_cc_out = nc.dram_tensor("h1_cc_out", [NK, 128], gate_dt, addr_space="Shared")
    h2_cc_in = nc.dram_tensor("h2_cc_in", [1, 128], wout_dt)
    h2_cc_out = nc.dram_tensor("h2_cc_out", [NK, 128], wout_dt, addr_space="Shared")
    s_cc_in = nc.dram_tensor("s_cc_in", [1, 8], f32)
    s_cc_out = nc.dram_tensor("s_cc_out", [8, 8], f32, addr_space="Shared")

    RG = [list(range(N_CORES))]

    with tile.TileContext(nc) as tc:
        with (
            tc.tile_pool(name="sp", bufs=1) as sp,
            tc.tile_pool(name="wg", bufs=8) as wg,
            tc.tile_pool(name="wo", bufs=10) as wo,
            tc.tile_pool(name="ps", bufs=1, space="PSUM") as ps,
            tc.tile_pool(name="pslg", bufs=4, space="PSUM") as pslg,
        ):
            # ---------- embedding row fetch + ReLU ----------
            widx_reg = nc.alloc_registers(
                "widx_reg", bass.OrderedSet([mybir.EngineType.Pool])
            )
            nc.reg_load(widx_reg.handles[0], widx.ap()[0:1, 0:1])
            off = nc.snap(widx_reg, donate=True, min_val=0, max_val=VOCAB - 1)

            emb3 = emb.ap().rearrange("v (m p) -> v m p", p=128)
            x_raw = sp.tile([128, NK], f32, name="x_raw")
            # src [1, NK, 128] iterated (m, p); dst iterated (p, m) -> transpose AP
            nc.gpsimd.dma_start(
                x_raw, emb3[bass.ds(off, 1), :, :].rearrange("o m p -> (o p) m")
            )

            # xh0x = relu(x); xh0h = h0[0] hi (cols 0..7) + lo (cols 8..15)
            xh0x = sp.tile([128, NK], gate_dt, name="xh0x")
            nc.scalar.activation(xh0x, x_raw, AF.Relu)
            xh0h = sp.tile([128, 2 * NK], gate_dt, name="xh0h")
            nc.gpsimd.dma_start(xh0h, h0a.ap().rearrange("m p -> p m"))

            b0_sb = sp.tile([1, GATE], f32, name="b0_sb")
            nc.gpsimd.dma_start(b0_sb, b0.ap())
            b1_sb = sp.tile([1, GATE], f32, name="b1_sb")
            nc.gpsimd.dma_start(b1_sb, b1.ap())
            c0a_sb = sp.tile([1, 128], f32, name="c0a_sb")
            nc.gpsimd.dma_start(c0a_sb, c0a.ap())
            c0b_sb = sp.tile([1, 128], f32, name="c0b_sb")
            nc.gpsimd.dma_start(c0b_sb, c0b.ap())
            bout_sb = sp.tile([1, VLOC], f32, name="bout_sb")
            nc.gpsimd.dma_start(bout_sb, bout.ap())

            def lstm_layer(
                lhs_cols, wT, b_sb, c_prev, h_out_d, c_out_d, h_cc_in_d, send_lo, tag
            ):
                """Gate matmuls + nonlinearity for one layer (this core's
                128-slice of the hidden state).  lhs_cols[m] = list of lhsT
                column APs (hi, and optionally lo) for weight k-chunk m.
                Gate row order is [i, f, o, g] (one fused sigmoid).  The
                h slice is cast to fp16 (hi, + lo residual when send_lo) and
                written to h_cc_in_d for the AllGather."""
                g_ps = ps.tile([1, GATE], f32, name=f"g_ps{tag}")
                blocks = []
                for q in range(4):
                    wt = wg.tile([128, 4 * GATE], gate_dt, name="wt_g", tag="wt_g")
                    nc.sync.dma_start(wt, wT.ap()[q])
                    blocks.append(wt)
                n_mm = sum(len(c) for c in lhs_cols)
                i_mm = 0
                for m in range(2 * NK):
                    wslc = blocks[m // 4][:, (m % 4) * GATE : (m % 4 + 1) * GATE]
                    for col in lhs_cols[m]:
                        nc.tensor.matmul(
                            g_ps,
                            col,
                            wslc,
                            start=(i_mm == 0),
                            stop=(i_mm == n_mm - 1),
                        )
                        i_mm += 1
                gates = sp.tile([1, GATE], f32, name=f"gates{tag}")
                nc.vector.tensor_add(gates, g_ps, b_sb)
                act = sp.tile([1, GATE], f32, name=f"act{tag}")
                nc.scalar.activation(act[:, 0:384], gates[:, 0:384], AF.Sigmoid)
                nc.scalar.activation(act[:, 384:512], gates[:, 384:512], AF.Tanh)
                t_fc = sp.tile([1, 128], f32, name=f"t_fc{tag}")
                nc.vector.tensor_mul(t_fc, act[:, 128:256], c_prev)
                t_ig = sp.tile([1, 128], f32, name=f"t_ig{tag}")
                nc.vector.tensor_mul(t_ig, act[:, 0:128], act[:, 384:512])
                c_new = sp.tile([1, 128], f32, name=f"c_new{tag}")
                nc.vector.tensor_add(c_new, t_fc, t_ig)
                tanh_c = sp.tile([1, 128], f32, name=f"tanh_c{tag}")
                nc.scalar.activation(tanh_c, c_new, AF.Tanh)
                h_new = sp.tile([1, 128], f32, name=f"h_new{tag}")
                nc.vector.tensor_mul(h_new, act[:, 256:384], tanh_c)
                npay = 256 if send_lo else 128
                hpay = sp.tile([1, npay], wout_dt if not send_lo else gate_dt,
                               name=f"hpay{tag}")
                nc.vector.tensor_copy(hpay[:, 0:128], h_new)
                if send_lo:
                    hi32 = sp.tile([1, 128], f32, name=f"hi32{tag}")
                    nc.vector.tensor_copy(hi32, hpay[:, 0:128])
                    nc.vector.tensor_sub(hpay[:, 128:256], h_new, hi32)
                nc.scalar.dma_start(h_cc_in_d.ap(), hpay)
                nc.gpsimd.dma_start(c_out_d.ap(), c_new)
                nc.gpsimd.dma_start(h_out_d.ap(), h_new)
                return h_new, c_new

            # ---------- layer 0 ----------
            lhs0 = [[xh0x[:, m : m + 1]] for m in range(NK)] + [
                [xh0h[:, m : m + 1], xh0h[:, m + NK : m + NK + 1]] for m in range(NK)
            ]
            lstm_layer(lhs0, w0T, b0_sb, c0a_sb, h1_out, c1_out, h1_cc_in, False, "0")
            nc.gpsimd.collective_compute(
                "AllGather",
                mybir.AluOpType.bypass,
                replica_groups=RG,
                ins=[h1_cc_in.ap()],
                outs=[h1_cc_out.ap()],
            )

            # ---------- layer 1 ----------
            # xh1 cols: 0..7 h1 (fp16); 8..15 h0[1] hi (lo pass dropped --
            # layer-1 inputs ride single fp16, error stays ~2e-4)
            xh1 = sp.tile([128, 2 * NK], gate_dt, name="xh1")
            nc.scalar.dma_start(xh1[:, 0:NK], h1_cc_out.ap().rearrange("m p -> p m"))
            nc.gpsimd.dma_start(
                xh1[:, NK : 2 * NK], h0b.ap()[0:NK, :].rearrange("m p -> p m")
            )
            lhs1 = [[xh1[:, m : m + 1]] for m in range(2 * NK)]
            lstm_layer(lhs1, w1T, b1_sb, c0b_sb, h2_out, c2_out, h2_cc_in, False, "1")
            nc.gpsimd.collective_compute(
                "AllGather",
                mybir.AluOpType.bypass,
                replica_groups=RG,
                ins=[h2_cc_in.ap()],
                outs=[h2_cc_out.ap()],
            )

            # full h2 (fp16) -> [128, 8], chunk m in col m
            h2c = sp.tile([128, NK], wout_dt, name="h2c")
            nc.scalar.dma_start(h2c, h2_cc_out.ap().rearrange("m p -> p m"))

            # ---------- logits matvec + exp partial sums ----------
            logits = sp.tile([1, VLOC], f32, name="logits")
            s_parts = sp.tile([1, NT], f32, name="s_parts")
            for t in range(NT):
                n0 = t * 512
                nn = min(512, VLOC - n0)
                wt = wo.tile([128, NK * 512], wout_dt, name="wt_o", tag="wt_o")
                eng = nc.sync
                eng.dma_start(wt[:, : NK * nn], woutT.ap()[t, :, : NK * nn])
                lg_ps = pslg.tile([1, 512], f32, name="lg_ps", tag="lg")
                for m in range(NK):
                    nc.tensor.matmul(
                        lg_ps[:, :nn],
                        h2c[:, m : m + 1],
                        wt[:, m * nn : (m + 1) * nn],
                        start=(m == 0),
                        stop=(m == NK - 1),
                    )
                nc.vector.tensor_add(
                    logits[:, n0 : n0 + nn], lg_ps[:, :nn], bout_sb[:, n0 : n0 + nn]
                )
                exp_scr = sp.tile([1, 512], f32, name="exp_scr", tag="exp_scr", bufs=2)
                nc.scalar.activation(
                    exp_scr[:, :nn],
                    logits[:, n0 : n0 + nn],
                    AF.Exp,
                    accum_out=s_parts[:, t : t + 1],
                )

            # ---------- sharded log-softmax stats ----------
            s_sum = sp.tile([1, 1], f32, name="s_sum")
            nc.vector.reduce_sum(s_sum, s_parts, axis=mybir.AxisListType.X)
            s_bcast = sp.tile([1, 8], f32, name="s_bcast")
            nc.vector.tensor_copy(s_bcast, s_sum.to_broadcast((1, 8)))
            nc.scalar.dma_start(s_cc_in.ap(), s_bcast)
            nc.gpsimd.collective_compute(
                "AllGather",
                mybir.AluOpType.bypass,
                replica_groups=RG,
                ins=[s_cc_in.ap()],
                outs=[s_cc_out.ap()],
            )
            s_col = sp.tile([8, 1], f32, name="s_col")
            nc.scalar.dma_start(s_col, s_cc_out.ap()[:, 0:1])
            ones8 = sp.tile([8, 128], f32, name="ones8")
            nc.vector.memset(ones8, 1.0)
            S_ps = ps.tile([128, 1], f32, name="S_ps")
            nc.tensor.matmul(S_ps, ones8, s_col, start=True, stop=True)
            logS = sp.tile([1, 1], f32, name="logS")
            nc.scalar.activation(logS, S_ps[0:1, :], AF.Ln)
            neglogS = sp.tile([1, 1], f32, name="neglogS")
            nc.scalar.mul(neglogS, logS, -1.0)

            # logp = logits - logS (split across ACT and DVE), in-place
            half = 2816
            nc.scalar.activation(
                logits[:, 0:half], logits[:, 0:half], AF.Identity, bias=neglogS
            )
            nc.vector.tensor_scalar_add(logits[:, half:], logits[:, half:], neglogS)
            nc.scalar.dma_start(logp_out.ap(), logits)

    nc.compile()
    return nc


def _get_program():
    if "nc" not in _CACHE:
        _CACHE["nc"] = _build_program()
    return _CACHE["nc"]


def _pack_gate_w(Wih_rows, Whh_rows, gate_np):
    """[GATE, HID] x2 -> [4, 128, 4*GATE] pre-tiled blocks: block q holds
    k-chunks 4q..4q+3 of [Wih.T ; Whh.T] side by side."""
    wT = np.concatenate([Wih_rows.T, Whh_rows.T], axis=0).astype(gate_np)  # [2H, GATE]
    out = np.zeros((4, 128, 4 * GATE), dtype=gate_np)
    for m in range(16):
        out[m // 4, :, (m % 4) * GATE : (m % 4 + 1) * GATE] = wT[
            m * 128 : (m + 1) * 128
        ]
    return out


def _hilo_chunks(v, gate_np):
    """[HID] f32 -> [2*NK, 128]: rows 0..NK-1 = hi chunks, NK.. = lo chunks."""
    hi = v.astype(gate_np)
    lo = (v - hi.astype(np.float32)).astype(gate_np)
    return np.ascontiguousarray(
        np.concatenate([hi.reshape(NK, 128), lo.reshape(NK, 128)], axis=0)
    )


def _prep_inputs(inputs):
    """Host-side sharding: slice + transpose weights per core."""
    wout_np = np.float16 if WOUT_HALF else np.float32
    gate_np = np.float16 if GATE_HALF else np.float32

    f = lambda k: np.asarray(inputs[k], dtype=np.float32)
    emb = f("emb")
    h0, c0 = f("h0"), f("c0")
    W_ih = [f("W_ih0"), f("W_ih1")]
    W_hh = [f("W_hh0"), f("W_hh1")]
    b = [
        (f("b_ih0") + f("b_hh0")).astype(np.float32),
        (f("b_ih1") + f("b_hh1")).astype(np.float32),
    ]
    W_out, b_out = f("W_out"), f("b_out")
    idx = np.uint32(np.asarray(inputs["word_input"]).reshape(-1)[0])

    in_maps = []
    for j in range(N_CORES):
        rows = np.concatenate(
            [np.arange(g * HID + j * 128, g * HID + (j + 1) * 128) for g in (0, 1, 3, 2)]
        )
        w0T = _pack_gate_w(W_ih[0][rows], W_hh[0][rows], gate_np)
        w1T = _pack_gate_w(W_ih[1][rows], W_hh[1][rows], gate_np)
        v0 = j * VLOC
        nvalid = max(0, min(VLOC, VOCAB - v0))
        wsh = np.zeros((HID, VLOC), dtype=wout_np)
        wsh[:, :nvalid] = W_out[v0 : v0 + nvalid].T.astype(wout_np)
        woutT = np.zeros((NT, 128, NK * 512), dtype=wout_np)
        for t in range(NT):
            nn = min(512, VLOC - t * 512)
            for m in range(NK):
                woutT[t, :, m * nn : (m + 1) * nn] = wsh[
                    m * 128 : (m + 1) * 128, t * 512 : t * 512 + nn
                ]
        boutj = np.full((1, VLOC), PAD_BIAS, dtype=np.float32)
        boutj[0, :nvalid] = b_out[v0 : v0 + nvalid]
        in_maps.append(
            {
                "emb": emb,
                "widx": np.array([[idx]], dtype=np.uint32),
                "h0a": _hilo_chunks(h0[0].reshape(HID), gate_np),
                "h0b": _hilo_chunks(h0[1].reshape(HID), gate_np),
                "c0a": np.ascontiguousarray(c0[0, 0, j * 128 : (j + 1) * 128]).reshape(
                    1, 128
                ),
                "c0b": np.ascontiguousarray(c0[1, 0, j * 128 : (j + 1) * 128]).reshape(
                    1, 128
                ),
                "w0T": w0T,
                "w1T": w1T,
                "b0": b[0][rows].reshape(1, GATE).copy(),
                "b1": b[1][rows].reshape(1, GATE).copy(),
                "woutT": woutT,
                "bout": boutj,
            }
        )
    return in_maps


def _assemble(results):
    h1 = np.concatenate([r["h1_out"].reshape(128) for r in results])
    h2 = np.concatenate([r["h2_out"].reshape(128) for r in results])
    c1 = np.concatenate([r["c1_out"].reshape(128) for r in results])
    c2 = np.concatenate([r["c2_out"].reshape(128) for r in results])
    logp = np.concatenate([r["logp_out"].reshape(VLOC) for r in results])[:VOCAB]
    h_out = np.stack([h1, h2]).reshape(2, 1, HID).astype(np.float32)
    c_out = np.stack([c1, c2]).reshape(2, 1, HID).astype(np.float32)
    return logp.reshape(1, VOCAB).astype(np.float32), (h_out, c_out)


def kernel(**inputs):
    from concourse.bass_utils import run_bass_kernel_spmd

    nc = _get_program()
    in_maps = _prep_inputs(inputs)
    res = run_bass_kernel_spmd(nc, in_maps, core_ids=list(range(N_CORES)))
    return _assemble(res.results)
